# revision 8
# baseline (speedup 1.0000x reference)
"""Bass/Trainium2 kernel for ComplexUpSampling2D (2x bilinear, half-pixel centers).

Input:  (16, 128, 128, 128) f32  (B, H, W, C)
Output: (16, 256, 256, 128) f32

Math (per axis, factor 2, half-pixel, with edge clamp):
  out[2i]   = 0.25*in[i-1] + 0.75*in[i]    (in[-1] clamped to in[0])
  out[2i+1] = 0.75*in[i]   + 0.25*in[i+1]  (in[n] clamped to in[n-1])

Strategy (pure data-parallel over batch: 2 images per core on 8 cores):
  - SBUF layout: partitions = H (128), free dim = W*C (16384) per image,
    processed in free-dim chunks of F with a one-w-block halo each side.
  - H-interp mixes partitions -> partition-shifted copies (prv/nxt) of the
    raw chunk via small SBUF->SBUF DMAs on gpsimd (SWDGE).
  - W-interp mixes w-neighbors C elements apart in the free dim -> plain
    shifted access patterns on the halo'd tiles.
  - All weighted sums are single fused scalar_tensor_tensor DVE ops with
    fp32-exact weights:
        curq3 = cur * (3/16)                      (scalar engine)
        qE    = prv * (1/16) + curq3              -> out row 2p   (= row/4)
        qO    = nxt * (1/16) + curq3              -> out row 2p+1
        out[., even w] = 3*q[j] + q[j-1]
        out[., odd  w] = 3*q[j] + q[j+1]
  - Raw bass with explicit semaphores (the walrus codegen on this run path
    supports only one embedded sync-wait per instruction, so all waits are
    standalone wait_ge ops).
  - DMA semaphores are parity-split so that every wait threshold equals
    16 x (all DMAs ever issued on that semaphore at that point): a DMA's
    16 completion increments are spread across engines, so a shared-sem
    cumulative wait can otherwise be satisfied by partial credit from a
    later in-flight DMA.
  - All semaphores are reset to zero at the end behind a finish barrier so
    the NEFF can be re-executed.
"""

from contextlib import ExitStack

import numpy as np

import concourse.bass as bass
from concourse import mybir
from concourse.bass_utils import run_bass_kernel_spmd

B, H, W, C = 16, 128, 128, 128
NCORES = 8
BS = B // NCORES          # images per core
WC = W * C                # 16384 free elements per input row
F = 2048                  # chunk width (input free elements) = 16 w-blocks
NW = F // C               # w-blocks per chunk
NCH = WC // F             # chunks per image
TOT = BS * NCH            # chunks per core
EXT = F + 2 * C           # chunk + one w-block halo on each side
NBUF = 2                  # double buffering (parity sems assume NBUF == 2)

_FP = mybir.dt.float32
_MUL = mybir.AluOpType.mult
_ADD = mybir.AluOpType.add


def _chunks():
    return [(b * NCH + k, b, k) for b in range(BS) for k in range(NCH)]


def _n_in_dmas(k):
    return 2 if (k == 0 or k == NCH - 1) else 1


def _build(**bass_kwargs):
    nc = bass.Bass(**bass_kwargs)
    x = nc.dram_tensor("x", [BS, H, WC], _FP, kind="ExternalInput")
    y = nc.dram_tensor("y", [BS, 2 * H, 2 * WC], _FP, kind="ExternalOutput")

    chunks = _chunks()
    # per-parity cumulative in-DMA counts AFTER chunk ci
    in_par = [0, 0]
    in_cum_par = []     # value of in_par[ci % 2] after chunk ci's loads
    for ci, b, k in chunks:
        in_par[ci % 2] += _n_in_dmas(k)
        in_cum_par.append(in_par[ci % 2])

    def sh_cum(ci):     # shift DMAs on parity sem after chunk ci: 4 per chunk
        return 4 * (ci // 2 + 1)

    def out_cum(ci):    # store DMAs on parity sem after chunk ci: 2 per chunk
        return 2 * (ci // 2 + 1)

    with ExitStack() as ctx:
        def sb(nm, wide=False):
            return ctx.enter_context(
                nc.sbuf_tensor(nm, [128, 2 * F if wide else EXT], _FP)
            )

        cur = [sb(f"cur{i}") for i in range(NBUF)]
        curq = [sb(f"curq{i}") for i in range(NBUF)]
        prv = [sb(f"prv{i}") for i in range(NBUF)]
        nxt = [sb(f"nxt{i}") for i in range(NBUF)]
        qe = [sb(f"qe{i}") for i in range(NBUF)]
        qo = [sb(f"qo{i}") for i in range(NBUF)]
        oute = [sb(f"oute{i}", wide=True) for i in range(NBUF)]
        outo = [sb(f"outo{i}", wide=True) for i in range(NBUF)]

        sem = lambda nm: ctx.enter_context(nc.semaphore(nm))
        s_in = [sem("s_in0"), sem("s_in1")]
        s_sh = [sem("s_sh0"), sem("s_sh1")]
        s_out = [sem("s_out0"), sem("s_out1")]
        s_act = sem("s_act")
        s_dve = sem("s_dve")
        s_fin = sem("s_fin")
        all_sems = s_in + s_sh + s_out + [s_act, s_dve, s_fin]

        block = ctx.enter_context(nc.Block())

        @block.sync
        def _(sync):
            for ci, b, k in chunks:
                p = ci % 2
                if ci >= NBUF:
                    # cur[p] readers from chunk ci-2 must be done
                    sync.wait_ge(s_act, ci - 1)
                    sync.wait_ge(s_dve, 6 * (ci - 2) + 2)  # gp shifts read cur too,
                    # but gp(ci-2) completion is implied by DVE having consumed
                    # prv/nxt(ci-2); still, gp itself must have ISSUED them:
                    sync.wait_ge(s_sh[p], 16 * sh_cum(ci - 2))
                xb = x[b]
                lo = k * F - C
                if k == 0:
                    sync.dma_start(out=cur[p][:, C:EXT], in_=xb[:, 0 : F + C]).then_inc(s_in[p], 16)
                    sync.dma_start(out=cur[p][:, 0:C], in_=xb[:, 0:C]).then_inc(s_in[p], 16)
                elif k == NCH - 1:
                    sync.dma_start(out=cur[p][:, 0 : F + C], in_=xb[:, lo:WC]).then_inc(s_in[p], 16)
                    sync.dma_start(out=cur[p][:, F + C : EXT], in_=xb[:, WC - C : WC]).then_inc(s_in[p], 16)
                else:
                    sync.dma_start(out=cur[p][:, :], in_=xb[:, lo : lo + EXT]).then_inc(s_in[p], 16)
                if ci >= 1:
                    pci, pb, pk = chunks[ci - 1]
                    pp = pci % 2
                    sync.wait_ge(s_dve, 6 * pci + 6)
                    yb = y[pb]
                    sync.dma_start(
                        out=yb[0 : 2 * H : 2, 2 * pk * F : 2 * (pk + 1) * F],
                        in_=oute[pp][:],
                    ).then_inc(s_out[pp], 16)
                    sync.dma_start(
                        out=yb[1 : 2 * H : 2, 2 * pk * F : 2 * (pk + 1) * F],
                        in_=outo[pp][:],
                    ).then_inc(s_out[pp], 16)
            # final chunk's stores
            ci, b, k = chunks[-1]
            p = ci % 2
            sync.wait_ge(s_dve, 6 * ci + 6)
            yb = y[b]
            sync.dma_start(
                out=yb[0 : 2 * H : 2, 2 * k * F : 2 * (k + 1) * F], in_=oute[p][:]
            ).then_inc(s_out[p], 16)
            sync.dma_start(
                out=yb[1 : 2 * H : 2, 2 * k * F : 2 * (k + 1) * F], in_=outo[p][:]
            ).then_inc(s_out[p], 16)
            # ---- finish: wait all stores landed, all engines idle, reset sems
            sync.wait_ge(s_out[0], 16 * out_cum(TOT - 2 + (TOT % 2)))
            sync.wait_ge(s_out[1], 16 * out_cum(TOT - 1 - (TOT % 2)))
            sync.wait_ge(s_fin, 2)
            for s in all_sems:
                sync.sem_clear(s)

        @block.scalar
        def _(act):
            # The scalar engine both issues the partition-shift DMAs (on its
            # own HWDGE ring, qActDynamicHW — the gpsimd SWDGE path serializes
            # on Q7 descriptor generation) and computes curq3.
            for ci, b, k in chunks:
                p = ci % 2
                act.wait_ge(s_in[p], 16 * in_cum_par[ci])
                if ci >= NBUF:
                    # prv/nxt/curq[p] readers (qE,qO of chunk ci-2) must be done
                    act.wait_ge(s_dve, 6 * (ci - 2) + 2)
                act.dma_start(out=prv[p][1:128, :], in_=cur[p][0:127, :]).then_inc(s_sh[p], 16)
                act.dma_start(out=prv[p][0:1, :], in_=cur[p][0:1, :]).then_inc(s_sh[p], 16)
                act.dma_start(out=nxt[p][0:127, :], in_=cur[p][1:128, :]).then_inc(s_sh[p], 16)
                act.dma_start(out=nxt[p][127:128, :], in_=cur[p][127:128, :]).then_inc(s_sh[p], 16)
                act.activation(
                    curq[p][:], cur[p][:], mybir.ActivationFunctionType.Copy,
                    scale=0.1875,
                ).then_inc(s_act, 1)
            act.sem_inc(s_fin, 1)

        @block.vector
        def _(vec):
            for ci, b, k in chunks:
                p = ci % 2
                vec.wait_ge(s_act, ci + 1)
                vec.wait_ge(s_sh[p], 16 * sh_cum(ci))
                vec.scalar_tensor_tensor(
                    qe[p][:], prv[p][:], 0.0625, curq[p][:], _MUL, _ADD
                ).then_inc(s_dve, 1)
                vec.scalar_tensor_tensor(
                    qo[p][:], nxt[p][:], 0.0625, curq[p][:], _MUL, _ADD
                ).then_inc(s_dve, 1)
                if ci >= NBUF:
                    vec.wait_ge(s_out[p], 16 * out_cum(ci - 2))
                qev = qe[p][:].rearrange("p (a c) -> p a c", c=C)
                qov = qo[p][:].rearrange("p (a c) -> p a c", c=C)
                ev = oute[p][:].rearrange("p (a t c) -> p a t c", t=2, c=C)
                ov = outo[p][:].rearrange("p (a t c) -> p a t c", t=2, c=C)
                vec.scalar_tensor_tensor(
                    ev[:, :, 0, :], qev[:, 1 : NW + 1, :], 3.0,
                    qev[:, 0:NW, :], _MUL, _ADD,
                ).then_inc(s_dve, 1)
                vec.scalar_tensor_tensor(
                    ev[:, :, 1, :], qev[:, 1 : NW + 1, :], 3.0,
                    qev[:, 2 : NW + 2, :], _MUL, _ADD,
                ).then_inc(s_dve, 1)
                vec.scalar_tensor_tensor(
                    ov[:, :, 0, :], qov[:, 1 : NW + 1, :], 3.0,
                    qov[:, 0:NW, :], _MUL, _ADD,
                ).then_inc(s_dve, 1)
                vec.scalar_tensor_tensor(
                    ov[:, :, 1, :], qov[:, 1 : NW + 1, :], 3.0,
                    qov[:, 2 : NW + 2, :], _MUL, _ADD,
                ).then_inc(s_dve, 1)
            vec.sem_inc(s_fin, 1)

    return nc


_NC = None


def kernel(inputs: np.ndarray) -> np.ndarray:
    global _NC
    assert inputs.shape == (B, H, W, C), inputs.shape
    x = np.ascontiguousarray(inputs, dtype=np.float32).reshape(B, H, WC)
    if _NC is None:
        _NC = _build()
    in_maps = [{"x": x[i * BS : (i + 1) * BS]} for i in range(NCORES)]
    res = run_bass_kernel_spmd(_NC, in_maps, list(range(NCORES))).results
    out = np.empty((B, 2 * H, 2 * W, C), dtype=np.float32)
    for i in range(NCORES):
        out[i * BS : (i + 1) * BS] = res[i]["y"].reshape(BS, 2 * H, 2 * W, C)
    return out
